# revision 37
# baseline (speedup 1.0000x reference)
"""GCN message-passing kernel for 8 Trainium2 NeuronCores.

Strategy: shard CHANNELS across the 8 cores (C=1280 -> 160 ch/core). Each core
computes the full output for its channel slice with zero collectives:
  - edge-encoder MLP: gamma/beta only for this core's 160 channels (W2 column
    shard), with the full h = relu(pose @ W1 + b1) recomputed per core on PE.
  - edges are host-sorted by dst and padded per 128-node dst window; the
    scatter-sum is a one-hot matmul on PE accumulating into PSUM per window.
  - the per-edge gather image[src] is an indirect DMA from an HBM-resident
    per-core image slice laid out [node, hw, ch] in bf16.
  - mean = PSUM evacuation with per-partition scale 1/cnt (0 for empty nodes).

All matmul/vector math in bf16 with f32 PSUM accumulation.
"""

import sys

sys.path.insert(0, "/opt/trn_rl_repo")

import numpy as np
import ml_dtypes

import concourse.bass as bass
import concourse.mybir as mybir
from concourse.tile import TileContext
from concourse.bass_utils import run_bass_kernel_spmd

BF16 = ml_dtypes.bfloat16
FP8 = ml_dtypes.float8_e4m3
P = 128
N_CORES = 8
CH_EDGES = 512  # edges per pipeline chunk
BPC = CH_EDGES // P  # blocks per chunk


def _split_excess_waits(nc):
    """This walrus build only encodes 1 sem-wait per instruction; hoist extra
    waits onto same-engine NoOps placed just before (engines run in order)."""
    for bb in nc.main_func.blocks:
        new_insts = []
        for ins in bb.instructions:
            si = ins.sync_info
            limit = 1
            if si is not None and si.on_wait and len(si.on_wait) > limit:
                waits = list(si.on_wait)
                extra, keep = waits[:-limit], waits[-limit:]
                for k, w in enumerate(extra):
                    nop = mybir.InstNoOp(name=f"{ins.name}-ws-{k}", ins=[], outs=[])
                    nop.engine = ins.engine
                    nop.sync_info = mybir.SyncInfo(on_wait=[w], on_update=[])
                    new_insts.append(nop)
                si.on_wait = keep
            new_insts.append(ins)
        bb.instructions[:] = new_insts


def _host_prep(pose, image, W1, b1, W2, b2, src, dst):
    """Sort/pad edges by dst window, build per-core shards and onehot blocks."""
    E = pose.shape[0]
    Nn, C, H, Wsp = image.shape
    HW = H * Wsp
    CS = C // N_CORES
    F = CS * HW
    n_win = Nn // P

    src = np.asarray(src).astype(np.int64)
    dst = np.asarray(dst).astype(np.int64)

    order = np.argsort(dst, kind="stable")
    blk_edge = []  # [B, 128] edge id, -1 = pad
    blk_win = []
    for w in range(n_win):
        sel = order[(dst[order] >= w * P) & (dst[order] < (w + 1) * P)]
        nb = max(1, -(-len(sel) // P))
        for b in range(nb):
            seg = sel[b * P : (b + 1) * P]
            row = np.full(P, -1, np.int64)
            row[: len(seg)] = seg
            blk_edge.append(row)
            blk_win.append(w)
    while len(blk_edge) % BPC != 0:
        blk_edge.append(np.full(P, -1, np.int64))
        blk_win.append(n_win - 1)
    blk_edge = np.stack(blk_edge)  # [B, 128]
    B = len(blk_edge)
    Ep = B * P

    valid = blk_edge >= 0
    eids = np.where(valid, blk_edge, 0)

    # gather src per edge slot (pad -> node 0)
    blk_src = np.where(valid, src[eids], 0).astype(np.int32)  # [B,128]
    # onehot: [B, 128 edge, 128 local-node], zero row for pads
    loc = (np.where(valid, dst[eids], 0) - np.asarray(blk_win)[:, None] * P).astype(
        np.int64
    )
    oh = np.zeros((B, P, P), np.float32)
    bi, pi = np.nonzero(valid)
    oh[bi, pi, loc[bi, pi]] = 1.0
    oh = oh.astype(BF16)

    # poseT padded: [9, Ep]
    pose_pad = np.where(valid.reshape(-1, 1), pose[eids.reshape(-1)], 0.0)
    poseT = np.ascontiguousarray(pose_pad.T.astype(BF16))  # [9, Ep]

    b2_allzero = not np.any(b2)
    cnt = np.bincount(dst, minlength=Nn).astype(np.float32)
    recip = np.where(cnt > 0, 1.0 / np.maximum(cnt, 1.0), 0.0).astype(np.float32)
    recip_t = np.ascontiguousarray(recip.reshape(n_win, P).T)  # [P, n_win]

    KT = C // P
    b1t = np.ascontiguousarray(b1.astype(np.float32).reshape(KT, P).T)  # [P, KT]
    idx_t = np.ascontiguousarray(blk_src.T)  # [P, B]

    # a block is skippable if it has no real edges and its window has other
    # real work (zero onehot rows contribute nothing)
    n_real = valid.sum(axis=1)
    win_blocks = {}
    for b, w in enumerate(blk_win):
        win_blocks.setdefault(w, []).append(b)
    keep = []
    for w, bs in win_blocks.items():
        real = [b for b in bs if n_real[b] > 0]
        keep.extend(real if real else bs[:1])
    keep = set(keep)
    # first/last block per window (among kept blocks)
    first_blk = {}
    last_blk = {}
    for b, w in enumerate(blk_win):
        if b in keep:
            first_blk.setdefault(w, b)
            last_blk[w] = b

    shared = dict(
        poseT=poseT,
        w1=W1.astype(BF16),
        b1t=b1t,
        idx=idx_t,
        oh=oh,
        recip=recip_t,
    )
    in_maps = []
    for j in range(N_CORES):
        c0 = j * CS
        cols_g = [2 * (c0 + i) for i in range(CS)]
        cols_b = [2 * (c0 + i) + 1 for i in range(CS)]
        cols = cols_g + cols_b
        w2f8 = W2[:, cols].astype(FP8)  # [C, 2*CS]
        b2row = b2[cols].reshape(1, -1).astype(BF16)  # [1, 2*CS]
        img = (
            image[:, c0 : c0 + CS]
            .transpose(0, 2, 3, 1)
            .reshape(Nn, F)
            .astype(BF16)
        )  # [Nn, F] layout [n, hw, c]
        in_maps.append(dict(shared, w2f8=w2f8, b2row=b2row, image=img))

    meta = dict(
        E=E, Nn=Nn, C=C, HW=HW, CS=CS, F=F, n_win=n_win, B=B, Ep=Ep, KT=KT,
        blk_win=blk_win, first_blk=first_blk, last_blk=last_blk,
        b2_allzero=b2_allzero, keep=keep,
    )
    return in_maps, meta


def _build(meta):
    Nn, CS, F, HW = meta["Nn"], meta["CS"], meta["F"], meta["HW"]
    n_win, B, Ep, KT = meta["n_win"], meta["B"], meta["Ep"], meta["KT"]
    C = meta["C"]
    blk_win, first_blk, last_blk = (
        meta["blk_win"], meta["first_blk"], meta["last_blk"],
    )
    f32 = mybir.dt.float32
    bf16 = mybir.dt.bfloat16
    fp8 = mybir.dt.float8e4
    NCH = B // BPC
    seg_cols = [(s, min(512, F - s)) for s in range(0, F, 512)]
    assert KT % 2 == 0, "fp8 DoubleRow path needs an even k-tile count"

    nc = bass.Bass()
    poseT_d = nc.declare_dram_parameter("poseT", [9, Ep], bf16, isOutput=False)
    w1_d = nc.declare_dram_parameter("w1", [9, C], bf16, isOutput=False)
    b1t_d = nc.declare_dram_parameter("b1t", [P, KT], f32, isOutput=False)
    w2f8_d = nc.declare_dram_parameter("w2f8", [C, 2 * CS], fp8, isOutput=False)
    b2_d = nc.declare_dram_parameter("b2row", [1, 2 * CS], bf16, isOutput=False)
    img_d = nc.declare_dram_parameter("image", [Nn, F], bf16, isOutput=False)
    idx_d = nc.declare_dram_parameter("idx", [P, B], mybir.dt.int32, isOutput=False)
    oh_d = nc.declare_dram_parameter("oh", [B, P, P], bf16, isOutput=False)
    recip_d = nc.declare_dram_parameter("recip", [P, n_win], f32, isOutput=False)
    out_d = nc.declare_dram_parameter("out", [Nn, F], bf16, isOutput=True)

    with TileContext(nc) as tc:
        with (
            tc.tile_pool(name="const", bufs=1) as constp,
            tc.tile_pool(name="ht", bufs=3) as htp,
            tc.tile_pool(name="gb", bufs=2 * BPC) as gbp,
            tc.tile_pool(name="xg", bufs=8) as xp,
            tc.tile_pool(name="mm", bufs=6) as mp,
            tc.tile_pool(name="ohp", bufs=2 * BPC) as ohp,
            tc.tile_pool(name="outp", bufs=2) as outp,
            tc.tile_pool(name="pw", bufs=1, space="PSUM") as pwp,
            tc.tile_pool(name="pb", bufs=1, space="PSUM") as pbp,
            tc.tile_pool(name="ps", bufs=2, space="PSUM") as psp,
        ):
            # ---- preload constants (first chunk's pose slice first, so the
            # PE can start immediately; the rest streams behind it) ----
            w1_sb = constp.tile([9, C], bf16)
            nc.sync.dma_start(out=w1_sb[:], in_=w1_d[:])
            poseT_sb = constp.tile([9, Ep], bf16)
            nc.sync.dma_start(
                out=poseT_sb[:, :CH_EDGES], in_=poseT_d[:, :CH_EDGES]
            )
            nc.sync.dma_start(
                out=poseT_sb[:, CH_EDGES:], in_=poseT_d[:, CH_EDGES:]
            )
            b1_sb = constp.tile([P, KT], f32)
            nc.sync.dma_start(out=b1_sb[:], in_=b1t_d[:])
            w2_sb = constp.tile([P, KT * 2 * CS], fp8)
            for t in range(KT):
                nc.sync.dma_start(
                    out=w2_sb[:, t * 2 * CS : (t + 1) * 2 * CS],
                    in_=w2f8_d[t * P : (t + 1) * P, :],
                )
            b2_sb = constp.tile([1, 2 * CS], bf16)
            nc.sync.dma_start(out=b2_sb[:], in_=b2_d[:])
            idx_sb = constp.tile([P, B], mybir.dt.int32)
            nc.sync.dma_start(out=idx_sb[:], in_=idx_d[:])
            recip_sb = constp.tile([P, n_win], f32)
            nc.sync.dma_start(out=recip_sb[:], in_=recip_d[:])
            ones_sb = constp.tile([1, P], bf16)
            nc.gpsimd.memset(ones_sb[:], 1.0)

            psw = None
            pbt = None
            keep = meta["keep"]
            # scatter matmuls are emitted ~2 blocks late, interleaved between
            # the low-duty DoubleRow matmuls, to keep the PE array busy enough
            # that the HAM clock gate stays at full rate
            pending = []
            for ci in range(NCH):
                if not any(ci * BPC + bi in keep for bi in range(BPC)):
                    continue
                # ---- hT chunk: [C(part-tiles), CH_EDGES] = relu(W1.T posT + b1)
                # stored fp8 — consumed only by the DoubleRow eT matmul
                hT = htp.tile([P, KT * CH_EDGES], fp8, tag="ht")
                for t in range(KT):
                    for e0 in range(0, CH_EDGES, 512):
                        ew = min(512, CH_EDGES - e0)
                        ph = psp.tile([P, 512], f32, tag="ps")
                        nc.tensor.matmul(
                            out=ph[:, :ew],
                            lhsT=w1_sb[:, t * P : (t + 1) * P],
                            rhs=poseT_sb[
                                :, ci * CH_EDGES + e0 : ci * CH_EDGES + e0 + ew
                            ],
                            start=True,
                            stop=True,
                        )
                        # alternate PSUM evacuation between ACT and DVE so PE
                        # isn't rate-limited by one evac engine (HAM throttle)
                        dst = hT[
                            :, t * CH_EDGES + e0 : t * CH_EDGES + e0 + ew
                        ]
                        if (t + e0 // 512) % 2 == 0:
                            nc.scalar.activation(
                                dst,
                                ph[:, :ew],
                                mybir.ActivationFunctionType.Relu,
                                bias=b1_sb[:, t : t + 1],
                                scale=1.0,
                            )
                        else:
                            nc.vector.tensor_scalar(
                                out=dst,
                                in0=ph[:, :ew],
                                scalar1=b1_sb[:, t : t + 1],
                                scalar2=0.0,
                                op0=mybir.AluOpType.add,
                                op1=mybir.AluOpType.max,
                            )
                for bi in range(BPC):
                    b = ci * BPC + bi
                    if b not in keep:
                        continue
                    w = blk_win[b]
                    # ---- gamma/beta for this 128-edge block (fp8 DoubleRow:
                    # two 128-row k-tiles contracted per instruction)
                    pe_ps = psp.tile([P, 512], f32, tag="ps")
                    hT3 = hT.rearrange("p (t e) -> p t e", t=KT)
                    w23 = w2_sb.rearrange("p (t c) -> p t c", t=KT)
                    for t2 in range(KT // 2):
                        nc.tensor.matmul(
                            out=pe_ps[:, : 2 * CS],
                            lhsT=hT3[:, 2 * t2 : 2 * t2 + 2, bi * P : (bi + 1) * P],
                            rhs=w23[:, 2 * t2 : 2 * t2 + 2, :],
                            start=(t2 == 0),
                            stop=(t2 == KT // 2 - 1 and meta["b2_allzero"]),
                            perf_mode=mybir.MatmulPerfMode.DoubleRow,
                        )
                        if len(pending) >= 13:
                            pending.pop(0)()
                    if not meta["b2_allzero"]:
                        nc.tensor.matmul(
                            out=pe_ps[:, : 2 * CS],
                            lhsT=ones_sb[:1, :P],
                            rhs=b2_sb[:1, :],
                            start=False,
                            stop=True,
                        )
                    gb = gbp.tile([P, 2 * CS], bf16, tag="gb")
                    nc.scalar.activation(
                        gb[:], pe_ps[:, : 2 * CS], mybir.ActivationFunctionType.Sigmoid
                    )
                    # ---- gather X = image[src] for the block
                    X = xp.tile([P, F], bf16, tag="xg")
                    nc.gpsimd.indirect_dma_start(
                        out=X[:],
                        out_offset=None,
                        in_=img_d[:],
                        in_offset=bass.IndirectOffsetOnAxis(
                            ap=idx_sb[:, b : b + 1], axis=0
                        ),
                    )
                    # ---- m = gamma (bcast over hw) * X
                    m = mp.tile([P, F], bf16, tag="mm")
                    g_b = (
                        gb[:, 0:CS]
                        .rearrange("p (o c) -> p o c", o=1)
                        .to_broadcast([P, HW, CS])
                    )
                    nc.vector.tensor_tensor(
                        out=m.rearrange("p (o c) -> p o c", o=HW),
                        in0=X.rearrange("p (o c) -> p o c", o=HW),
                        in1=g_b,
                        op=mybir.AluOpType.mult,
                    )
                    # ---- one-hot scatter matmuls into the window PSUM
                    oht = ohp.tile([P, P], bf16, tag="oh")
                    nc.sync.dma_start(out=oht[:], in_=oh_d[b])
                    first = first_blk[w] == b
                    last = last_blk[w] == b
                    if first:
                        psw = pwp.tile([P, F], f32, tag="pw")
                        pbt = pbp.tile([P, CS], f32, tag="pb")

                    def seg_mm(s, width, psw=psw, oht=oht, m=m, first=first, last=last):
                        nc.tensor.matmul(
                            out=psw[:, s : s + width],
                            lhsT=oht[:],
                            rhs=m[:, s : s + width],
                            start=first,
                            stop=last,
                            skip_group_check=True,
                        )

                    def beta_mm(pbt=pbt, oht=oht, gb=gb, first=first, last=last):
                        nc.tensor.matmul(
                            out=pbt[:],
                            lhsT=oht[:],
                            rhs=gb[:, CS : 2 * CS],
                            start=first,
                            stop=last,
                            skip_group_check=True,
                        )

                    for s, width in seg_cols:
                        pending.append(lambda s=s, width=width, f=seg_mm: f(s, width))
                    pending.append(beta_mm)

                    def evac(psw=psw, pbt=pbt, w=w):
                        # ---- evacuate window: out = psw*recip + (beta_seg*recip)
                        # split by column halves across ACT and DVE so the PSUM
                        # window frees ~2x sooner (it gates the next window)
                        bs = outp.tile([P, CS], bf16, tag="bs")
                        nc.scalar.activation(
                            bs[:],
                            pbt[:],
                            mybir.ActivationFunctionType.Copy,
                            scale=recip_sb[:, w : w + 1],
                        )
                        HF = F // 2
                        HO = HW // 2
                        bs_b = bs.rearrange("p (o c) -> p o c", o=1)
                        of = outp.tile([P, F], bf16, tag="of")
                        of3 = of.rearrange("p (o c) -> p o c", o=HW)
                        psw3 = psw.rearrange("p (o c) -> p o c", o=HW)
                        om = outp.tile([P, HF], bf16, tag="om")
                        nc.scalar.activation(
                            om[:],
                            psw[:, :HF],
                            mybir.ActivationFunctionType.Copy,
                            scale=recip_sb[:, w : w + 1],
                        )
                        nc.vector.scalar_tensor_tensor(
                            out=of3[:, HO:, :],
                            in0=psw3[:, HO:, :],
                            scalar=recip_sb[:, w : w + 1],
                            in1=bs_b.to_broadcast([P, HO, CS]),
                            op0=mybir.AluOpType.mult,
                            op1=mybir.AluOpType.add,
                        )
                        nc.vector.tensor_tensor(
                            out=of3[:, :HO, :],
                            in0=om.rearrange("p (o c) -> p o c", o=HO),
                            in1=bs_b.to_broadcast([P, HO, CS]),
                            op=mybir.AluOpType.add,
                        )
                        nc.sync.dma_start(
                            out=out_d[w * P : (w + 1) * P, :], in_=of[:]
                        )

                    if last:
                        pending.append(evac)
            while pending:
                pending.pop(0)()

    _split_excess_waits(nc)
    return nc


def _run(inputs, trace=False, trace_kwargs=None):
    pose = np.asarray(inputs["pose"], np.float32)
    image = np.asarray(inputs["image"], np.float32)
    W1 = np.asarray(inputs["W1"], np.float32)
    b1 = np.asarray(inputs["b1"], np.float32)
    W2 = np.asarray(inputs["W2"], np.float32)
    b2 = np.asarray(inputs["b2"], np.float32)
    src = np.asarray(inputs["src"])
    dst = np.asarray(inputs["dst"])

    in_maps, meta = _host_prep(pose, image, W1, b1, W2, b2, src, dst)
    nc = _build(meta)
    kw = {}
    if trace:
        kw = dict(trace=True, trace_kwargs=trace_kwargs or {})
    res = run_bass_kernel_spmd(nc, in_maps, core_ids=list(range(N_CORES)), **kw)
    Nn, C, HW, CS = meta["Nn"], meta["C"], meta["HW"], meta["CS"]
    H = int(np.sqrt(HW))
    out = np.empty((Nn, C, H, HW // H), np.float32)
    for j in range(N_CORES):
        oc = np.asarray(res.results[j]["out"]).astype(np.float32)
        out[:, j * CS : (j + 1) * CS] = (
            oc.reshape(Nn, HW, CS).transpose(0, 2, 1).reshape(Nn, CS, H, HW // H)
        )
    return out, res


def kernel(**inputs) -> np.ndarray:
    out, _ = _run(inputs)
    return out
